# revision 4
# baseline (speedup 1.0000x reference)
# Trainium2 Bass kernel for nn_DetectionLoss (B=32, N=25200, M=200, C=80).
#
# Strategy: pure data-parallel over batch (4 batches per core, 8 cores).
# The reference only reads pred_bbox[:, :M] and pred_cls[:, :M], so only
# those slices are shipped to the device. Each core computes per-partition
# partial sums of the four loss terms; the host does the final (tiny)
# cross-core reduction and mean/lambda arithmetic in float64.
#
# Device layouts (per core, b in [0,4), anchor pair k in {0,1}):
#   pairs:  flat pair i=(b,n) with n=2p+k  ->  partition p in [0,100), col (b,k)
#   obj:    pred_obj[b, n], n = p*225 + j  ->  partition p in [0,112), col (b,j)
# All DMAs are affine strided views of the (sliced) input tensors.

import numpy as np

B, N, M, C = 32, 25200, 200, 80
NCORES = 8
BPC = B // NCORES          # 4 batches per core
KP = 2                     # anchors per (partition, batch)
P_PAIRS = M // KP          # 100 partitions for pair-space tiles
P_OBJ, F_OBJ = 112, 225    # 25200 = 112 * 225
EPS = 1e-7

_CACHED_NC = None


def _emit(nc, tc, mybir, pb, po, pc_, gb, mk, out):
    import concourse.bass as bass  # noqa: F401

    f32 = mybir.dt.float32
    Alu = mybir.AluOpType
    Act = mybir.ActivationFunctionType

    with tc.tile_pool(name="main", bufs=1) as pool:
        ACC = pool.tile([128, 8], f32)
        nc.vector.memset(ACC[:], 0.0)

        # ---------------- bbox GIoU term ----------------
        # PB packs pred (s=0) and gt (s=1) boxes: [p, s, j=b*2+k, comp]
        PB = pool.tile([P_PAIRS, 2, BPC * KP, 4], f32)
        nc.sync.dma_start(
            out=PB[:, 0].rearrange("p (b k) c -> p b k c", k=KP),
            in_=pb.ap().rearrange("b (p k) c -> p b k c", k=KP),
        )
        nc.sync.dma_start(
            out=PB[:, 1].rearrange("p (b k) c -> p b k c", k=KP),
            in_=gb.ap().rearrange("b (p k) c -> p b k c", k=KP),
        )
        cxcy = PB[:, :, :, 0:2]
        wh = PB[:, :, :, 2:4]
        C1 = pool.tile([P_PAIRS, 2, BPC * KP, 2], f32)
        C2 = pool.tile([P_PAIRS, 2, BPC * KP, 2], f32)
        # corners: c -/+ wh/2
        nc.vector.scalar_tensor_tensor(C1[:], wh, -0.5, cxcy, Alu.mult, Alu.add)
        nc.vector.scalar_tensor_tensor(C2[:], wh, 0.5, cxcy, Alu.mult, Alu.add)
        I1 = pool.tile([P_PAIRS, BPC * KP, 2], f32)
        I2 = pool.tile([P_PAIRS, BPC * KP, 2], f32)
        E1 = pool.tile([P_PAIRS, BPC * KP, 2], f32)
        E2 = pool.tile([P_PAIRS, BPC * KP, 2], f32)
        nc.vector.tensor_tensor(I1[:], C1[:, 0], C1[:, 1], Alu.max)
        nc.vector.tensor_tensor(I2[:], C2[:, 0], C2[:, 1], Alu.min)
        nc.vector.tensor_tensor(E1[:], C1[:, 0], C1[:, 1], Alu.min)
        nc.vector.tensor_tensor(E2[:], C2[:, 0], C2[:, 1], Alu.max)
        ID = pool.tile([P_PAIRS, BPC * KP, 2], f32)
        IDr = pool.tile([P_PAIRS, BPC * KP, 2], f32)
        ED = pool.tile([P_PAIRS, BPC * KP, 2], f32)
        nc.vector.tensor_sub(ID[:], I2[:], I1[:])
        nc.vector.tensor_relu(IDr[:], ID[:])
        nc.vector.tensor_sub(ED[:], E2[:], E1[:])
        inter = pool.tile([P_PAIRS, BPC * KP], f32)
        encl = pool.tile([P_PAIRS, BPC * KP], f32)
        nc.vector.tensor_mul(inter[:], IDr[:, :, 0], IDr[:, :, 1])
        nc.vector.tensor_mul(encl[:], ED[:, :, 0], ED[:, :, 1])
        A = pool.tile([P_PAIRS, 2, BPC * KP], f32)
        nc.vector.tensor_mul(A[:], PB[:, :, :, 2], PB[:, :, :, 3])
        asum = pool.tile([P_PAIRS, BPC * KP], f32)
        nc.vector.tensor_add(asum[:], A[:, 0], A[:, 1])
        U = pool.tile([P_PAIRS, BPC * KP], f32)
        nc.vector.scalar_tensor_tensor(U[:], inter[:], -1.0, asum[:], Alu.mult, Alu.add)
        Ue = pool.tile([P_PAIRS, BPC * KP], f32)
        Ur = pool.tile([P_PAIRS, BPC * KP], f32)
        nc.vector.tensor_scalar_add(Ue[:], U[:], EPS)
        nc.vector.reciprocal(Ur[:], Ue[:])
        # NOTE: tensor_tensor_reduce wedges the device (NRT_EXEC_UNIT_UNRECOVERABLE)
        # on this runtime; scalar_tensor_tensor's accum_out path works.
        t8a = pool.tile([P_PAIRS, BPC * KP], f32)
        nc.vector.scalar_tensor_tensor(
            t8a[:], inter[:], 1.0, Ur[:], Alu.mult, Alu.mult,
            accum_out=ACC[0:P_PAIRS, 0:1],
        )
        EmU = pool.tile([P_PAIRS, BPC * KP], f32)
        Ee = pool.tile([P_PAIRS, BPC * KP], f32)
        Er = pool.tile([P_PAIRS, BPC * KP], f32)
        nc.vector.tensor_sub(EmU[:], encl[:], U[:])
        nc.vector.tensor_scalar_add(Ee[:], encl[:], EPS)
        nc.vector.reciprocal(Er[:], Ee[:])
        t8b = pool.tile([P_PAIRS, BPC * KP], f32)
        nc.vector.scalar_tensor_tensor(
            t8b[:], EmU[:], 1.0, Er[:], Alu.mult, Alu.mult,
            accum_out=ACC[0:P_PAIRS, 1:2],
        )

        # ---------------- objectness term ----------------
        # softplus(x) = Ln(Exp(x) + 1); fine in f32 for |x| ~ randn range.
        OB = pool.tile([P_OBJ, BPC, F_OBJ], f32)
        nc.sync.dma_start(
            out=OB[:], in_=po.ap().rearrange("b (p j) -> p b j", p=P_OBJ)
        )
        OBf = OB.rearrange("p b j -> p (b j)")
        Eo = pool.tile([P_OBJ, BPC * F_OBJ], f32)
        So = pool.tile([P_OBJ, BPC * F_OBJ], f32)
        nc.scalar.activation(Eo[:], OBf, Act.Exp)
        nc.scalar.activation(
            So[:], Eo[:], Act.Ln, bias=1.0, accum_out=ACC[0:P_OBJ, 2:3]
        )
        # positives (n < M): need softplus(-x), plus softplus(+x) to correct
        # the all-elements sum above (host subtracts).
        PT = pool.tile([P_PAIRS, BPC, KP], f32)
        nc.sync.dma_start(
            out=PT[:], in_=po.ap()[:, 0:M].rearrange("b (p k) -> p b k", k=KP)
        )
        PTf = PT.rearrange("p b k -> p (b k)")
        En = pool.tile([P_PAIRS, BPC * KP], f32)
        Sn = pool.tile([P_PAIRS, BPC * KP], f32)
        nc.scalar.activation(En[:], PTf, Act.Exp, scale=-1.0)
        nc.scalar.activation(
            Sn[:], En[:], Act.Ln, bias=1.0, accum_out=ACC[0:P_PAIRS, 3:4]
        )
        Ep = pool.tile([P_PAIRS, BPC * KP], f32)
        Sp = pool.tile([P_PAIRS, BPC * KP], f32)
        nc.scalar.activation(Ep[:], PTf, Act.Exp)
        nc.scalar.activation(
            Sp[:], Ep[:], Act.Ln, bias=1.0, accum_out=ACC[0:P_PAIRS, 4:5]
        )

        # ---------------- classification CE term ----------------
        CL = pool.tile([P_PAIRS, BPC * KP, C], f32)
        nc.sync.dma_start(
            out=CL.rearrange("p (b k) c -> p b k c", k=KP),
            in_=pc_.ap().rearrange("b (p k) c -> p b k c", k=KP),
        )
        MK = pool.tile([P_PAIRS, BPC * KP * C], f32)
        nc.sync.dma_start(out=MK[:], in_=mk.ap())
        CLf = CL.rearrange("p a c -> p (a c)")
        Ec = pool.tile([P_PAIRS, BPC * KP, C], f32)
        nc.scalar.activation(Ec.rearrange("p a c -> p (a c)"), CLf, Act.Exp)
        sums = pool.tile([P_PAIRS, BPC * KP], f32)
        nc.vector.reduce_sum(out=sums[:], in_=Ec[:], axis=mybir.AxisListType.X)
        lse = pool.tile([P_PAIRS, BPC * KP], f32)
        nc.scalar.activation(
            lse[:], sums[:], Act.Ln, accum_out=ACC[0:P_PAIRS, 5:6]
        )
        prod = pool.tile([P_PAIRS, BPC * KP * C], f32)
        nc.vector.scalar_tensor_tensor(
            prod[:], CLf, 1.0, MK[:], Alu.mult, Alu.mult,
            accum_out=ACC[0:P_PAIRS, 6:7],
        )

        nc.sync.dma_start(out=out.ap(), in_=ACC[:])


def build_bass():
    global _CACHED_NC
    if _CACHED_NC is not None:
        return _CACHED_NC
    import concourse.bacc as bacc
    import concourse.tile as tile
    import concourse.mybir as mybir

    f32 = mybir.dt.float32
    nc = bacc.Bacc("TRN2", target_bir_lowering=False, debug=False, num_devices=NCORES)
    pb = nc.dram_tensor("pred_bbox", [BPC, M, 4], f32, kind="ExternalInput")
    po = nc.dram_tensor("pred_obj", [BPC, N], f32, kind="ExternalInput")
    pc_ = nc.dram_tensor("pred_cls", [BPC, M, C], f32, kind="ExternalInput")
    gb = nc.dram_tensor("gt_boxes", [BPC, M, 4], f32, kind="ExternalInput")
    mk = nc.dram_tensor("mask", [P_PAIRS, BPC * KP * C], f32, kind="ExternalInput")
    out = nc.dram_tensor("partials", [128, 8], f32, kind="ExternalOutput")
    with tile.TileContext(nc) as tc:
        _emit(nc, tc, mybir, pb, po, pc_, gb, mk, out)
    nc.compile()
    _CACHED_NC = nc
    return nc


def make_in_maps(pred_bbox, pred_obj, pred_cls, gt_boxes, gt_labels):
    labels = np.asarray(gt_labels).astype(np.int64)
    in_maps = []
    for c in range(NCORES):
        bs = slice(c * BPC, (c + 1) * BPC)
        lab = labels[bs].reshape(BPC, P_PAIRS, KP)
        onehot = (lab[..., None] == np.arange(C)).astype(np.float32)
        mask = np.ascontiguousarray(onehot.transpose(1, 0, 2, 3)).reshape(
            P_PAIRS, BPC * KP * C
        )
        in_maps.append(
            {
                "pred_bbox": np.ascontiguousarray(pred_bbox[bs, :M]),
                "pred_obj": np.ascontiguousarray(pred_obj[bs]),
                "pred_cls": np.ascontiguousarray(pred_cls[bs, :M]),
                "gt_boxes": np.ascontiguousarray(gt_boxes[bs]),
                "mask": mask,
            }
        )
    return in_maps


def finalize(per_core_partials):
    S = np.zeros(8, np.float64)
    for p in per_core_partials:
        S += p.astype(np.float64).sum(axis=0)
    s_iou, s_ratio, s_all, s_pos, s_posplus, s_lse, s_picked = S[:7]
    n_pos = B * M
    n_neg = B * (N - M)
    loss_bbox = 5.0 * (n_pos - s_iou + s_ratio) / n_pos
    loss_obj = s_pos / n_pos + 0.5 * (s_all - s_posplus) / n_neg
    loss_cls = (s_lse - s_picked) / n_pos
    total = loss_bbox + loss_obj + loss_cls
    return np.array([total, loss_bbox, loss_obj, loss_cls], dtype=np.float32)


def kernel(pred_bbox, pred_obj, pred_cls, gt_boxes, gt_labels):
    from concourse.bass_utils import run_bass_kernel_spmd

    nc = build_bass()
    in_maps = make_in_maps(pred_bbox, pred_obj, pred_cls, gt_boxes, gt_labels)
    res = run_bass_kernel_spmd(nc, in_maps, core_ids=list(range(NCORES)))
    return finalize([r["partials"] for r in res.results])


# revision 6
# speedup vs baseline: 1.0847x; 1.0847x over previous
# Trainium2 Bass kernel for nn_DetectionLoss (B=32, N=25200, M=200, C=80).
#
# Strategy: pure data-parallel over batch (4 batches per core, 8 cores).
# The reference only reads pred_bbox[:, :M] and pred_cls[:, :M], so only
# those slices are shipped to the device. Each core computes per-partition
# partial sums of the four loss terms; the host does the final (tiny)
# cross-core reduction and mean/lambda arithmetic in float64.
#
# Device inputs per core (host-packed into device layout):
#   pairs [100, 1344]: cols 0:64 boxes (pred|gt interleaved per pair),
#                      64:704 cls logits, 704:1344 one-hot label mask
#   obj   [120, 900]:  rows 0:112 all 4*25200 obj logits (flat reshape),
#                      rows 112:116 -x of positives (softplus(-x) term),
#                      rows 116:120 +x of positives (correction term),
#                      positives rows padded with -30 (softplus(-30) == 0 in f32)
# Output per core: partials [128, 8] f32 of per-partition partial sums:
#   col 0 sum(iou), col 1 sum((enclose-union)/(enclose+eps)),
#   col 2 softplus sums (split by partition range as above),
#   col 3 sum(logsumexp), col 4 sum(picked logit)

import numpy as np

B, N, M, C = 32, 25200, 200, 80
NCORES = 8
BPC = B // NCORES          # 4 batches per core
KP = 2                     # anchors per (partition, batch)
P_PAIRS = M // KP          # 100 partitions for pair-space tiles
NPAIR = BPC * KP           # 8 pairs per partition
P_OBJ, F_OBJ = 112, 900    # 4*25200 = 112*900
EPS = 1e-7
PAD = -30.0                # softplus(PAD) == 0 exactly in f32

COL_PB, COL_CL, COL_MK = 0, 64, 704
W_PAIRS = 1344

_CACHED_NC = None


def _emit(nc, tc, mybir, pairs, obj, out):
    f32 = mybir.dt.float32
    Alu = mybir.AluOpType
    Act = mybir.ActivationFunctionType

    with tc.tile_pool(name="main", bufs=1) as pool:
        ACC = pool.tile([128, 8], f32, name="ACC")
        nc.vector.memset(ACC[:], 0.0)

        PAIRS = pool.tile([P_PAIRS, W_PAIRS], f32, name="PAIRS")
        OBJ = pool.tile([120, F_OBJ], f32, name="OBJ")
        # DMA queue split across the three DMA-capable engines
        nc.sync.dma_start(out=PAIRS[0:50], in_=pairs.ap()[0:50])
        nc.gpsimd.dma_start(out=PAIRS[50:100], in_=pairs.ap()[50:100])
        nc.scalar.dma_start(out=OBJ[0:60], in_=obj.ap()[0:60])
        nc.sync.dma_start(out=OBJ[60:120], in_=obj.ap()[60:120])

        # ---------------- objectness softplus (one fused tile) ----------------
        Eo = pool.tile([120, F_OBJ], f32, name="Eo")
        So = pool.tile([120, F_OBJ], f32, name="So")
        nc.scalar.activation(Eo[:], OBJ[:], Act.Exp)
        nc.scalar.activation(So[:], Eo[:], Act.Ln, bias=1.0,
                             accum_out=ACC[0:120, 2:3])

        # ---------------- classification CE term ----------------
        CLf = PAIRS[:, COL_CL:COL_CL + NPAIR * C]
        MKf = PAIRS[:, COL_MK:COL_MK + NPAIR * C]
        Ec = pool.tile([P_PAIRS, NPAIR, C], f32, name="Ec")
        nc.scalar.activation(Ec.rearrange("p a c -> p (a c)"), CLf, Act.Exp)
        sums = pool.tile([P_PAIRS, NPAIR], f32, name="sums")
        nc.vector.reduce_sum(out=sums[:], in_=Ec[:], axis=mybir.AxisListType.X)
        lse = pool.tile([P_PAIRS, NPAIR], f32, name="lse")
        nc.scalar.activation(lse[:], sums[:], Act.Ln,
                             accum_out=ACC[0:P_PAIRS, 3:4])
        prod = pool.tile([P_PAIRS, NPAIR * C], f32, name="prod")
        nc.vector.scalar_tensor_tensor(
            prod[:], CLf, 1.0, MKf, Alu.mult, Alu.mult,
            accum_out=ACC[0:P_PAIRS, 4:5],
        )

        # ---------------- bbox GIoU term ----------------
        # PB view [p, s(pred/gt), j(pair), c(cx,cy,w,h)]
        PB = PAIRS[:, COL_PB:COL_PB + 64].rearrange(
            "p (s j c) -> p s j c", s=2, c=4
        )
        cxcy = PB[:, :, :, 0:2]
        wh = PB[:, :, :, 2:4]
        C1 = pool.tile([P_PAIRS, 2, NPAIR, 2], f32, name="C1")
        C2 = pool.tile([P_PAIRS, 2, NPAIR, 2], f32, name="C2")
        nc.vector.scalar_tensor_tensor(C1[:], wh, -0.5, cxcy, Alu.mult, Alu.add)
        nc.vector.scalar_tensor_tensor(C2[:], wh, 0.5, cxcy, Alu.mult, Alu.add)
        I1 = pool.tile([P_PAIRS, NPAIR, 2], f32, name="I1")
        I2 = pool.tile([P_PAIRS, NPAIR, 2], f32, name="I2")
        E1 = pool.tile([P_PAIRS, NPAIR, 2], f32, name="E1")
        E2 = pool.tile([P_PAIRS, NPAIR, 2], f32, name="E2")
        nc.vector.tensor_tensor(I1[:], C1[:, 0], C1[:, 1], Alu.max)
        nc.vector.tensor_tensor(I2[:], C2[:, 0], C2[:, 1], Alu.min)
        nc.vector.tensor_tensor(E1[:], C1[:, 0], C1[:, 1], Alu.min)
        nc.vector.tensor_tensor(E2[:], C2[:, 0], C2[:, 1], Alu.max)
        ID = pool.tile([P_PAIRS, NPAIR, 2], f32, name="ID")
        IDr = pool.tile([P_PAIRS, NPAIR, 2], f32, name="IDr")
        ED = pool.tile([P_PAIRS, NPAIR, 2], f32, name="ED")
        nc.vector.tensor_sub(ID[:], I2[:], I1[:])
        nc.vector.tensor_relu(IDr[:], ID[:])
        nc.vector.tensor_sub(ED[:], E2[:], E1[:])
        inter = pool.tile([P_PAIRS, NPAIR], f32, name="inter")
        encl = pool.tile([P_PAIRS, NPAIR], f32, name="encl")
        nc.vector.tensor_mul(inter[:], IDr[:, :, 0], IDr[:, :, 1])
        nc.vector.tensor_mul(encl[:], ED[:, :, 0], ED[:, :, 1])
        A = pool.tile([P_PAIRS, 2, NPAIR], f32, name="A")
        nc.vector.tensor_mul(A[:], PB[:, :, :, 2], PB[:, :, :, 3])
        asum = pool.tile([P_PAIRS, NPAIR], f32, name="asum")
        nc.vector.tensor_add(asum[:], A[:, 0], A[:, 1])
        U = pool.tile([P_PAIRS, NPAIR], f32, name="U")
        nc.vector.scalar_tensor_tensor(U[:], inter[:], -1.0, asum[:],
                                       Alu.mult, Alu.add)
        Ue = pool.tile([P_PAIRS, NPAIR], f32, name="Ue")
        Ur = pool.tile([P_PAIRS, NPAIR], f32, name="Ur")
        nc.vector.tensor_scalar_add(Ue[:], U[:], EPS)
        nc.vector.reciprocal(Ur[:], Ue[:])
        # NOTE: tensor_tensor_reduce wedges the device (NRT_EXEC_UNIT_UNRECOVERABLE)
        # on this runtime; scalar_tensor_tensor's accum_out path works.
        t8a = pool.tile([P_PAIRS, NPAIR], f32, name="t8a")
        nc.vector.scalar_tensor_tensor(
            t8a[:], inter[:], 1.0, Ur[:], Alu.mult, Alu.mult,
            accum_out=ACC[0:P_PAIRS, 0:1],
        )
        EmU = pool.tile([P_PAIRS, NPAIR], f32, name="EmU")
        Ee = pool.tile([P_PAIRS, NPAIR], f32, name="Ee")
        Er = pool.tile([P_PAIRS, NPAIR], f32, name="Er")
        nc.vector.tensor_sub(EmU[:], encl[:], U[:])
        nc.vector.tensor_scalar_add(Ee[:], encl[:], EPS)
        nc.vector.reciprocal(Er[:], Ee[:])
        t8b = pool.tile([P_PAIRS, NPAIR], f32, name="t8b")
        nc.vector.scalar_tensor_tensor(
            t8b[:], EmU[:], 1.0, Er[:], Alu.mult, Alu.mult,
            accum_out=ACC[0:P_PAIRS, 1:2],
        )

        nc.sync.dma_start(out=out.ap(), in_=ACC[:])


def build_bass():
    global _CACHED_NC
    if _CACHED_NC is not None:
        return _CACHED_NC
    import concourse.bacc as bacc
    import concourse.tile as tile
    import concourse.mybir as mybir

    f32 = mybir.dt.float32
    Act = mybir.ActivationFunctionType

    class FastTileContext(tile.TileContext):
        # Same as TileContext._drain_and_barrier but: sem-only barrier and no
        # trailing second barrier — saves most of the ~9us kernel-tail cost.
        def _drain_and_barrier(self, tick_clock, wait_clock):
            drain_inst = self.nc.sync.drain()
            wait_clock.add_sem_waits(
                drain_inst.ins, tile.ScopedClock({None: tick_clock.global_clock})
            )
            self.nc.all_engine_barrier(sem_only=True)
            popped = self.nc._tile_sem_poison_stack.pop()
            assert popped is self._sem_poison
            self.nc.clear_and_free_semaphores(list(self.sems.allocated().values()))

    nc = bacc.Bacc("TRN2", target_bir_lowering=False, debug=False,
                   num_devices=NCORES)
    pairs = nc.dram_tensor("pairs", [P_PAIRS, W_PAIRS], f32, kind="ExternalInput")
    obj = nc.dram_tensor("obj", [120, F_OBJ], f32, kind="ExternalInput")
    out = nc.dram_tensor("partials", [128, 8], f32, kind="ExternalOutput")
    with FastTileContext(nc) as tc:
        _emit(nc, tc, mybir, pairs, obj, out)

    # Route every Exp/Ln to the one table that holds both, so the kernel pays
    # a single ACT_TABLE_LOAD instead of ping-ponging between per-func tables.
    # Patch is scoped to this compile; table ids are positional so only the
    # membership sets are altered (ids stay valid).
    orig_tables = bacc.get_activation_tables

    def _merged_tables(arch):
        out_d = {}
        for name, s in orig_tables(arch).items():
            s2 = set(s)
            if name != "natural_log_exp_and_others":
                s2.discard(Act.Exp)
                s2.discard(Act.Ln)
            out_d[name] = s2
        return out_d

    bacc.get_activation_tables = _merged_tables
    try:
        nc.compile()
    finally:
        bacc.get_activation_tables = orig_tables
    _CACHED_NC = nc
    return nc


def make_in_maps(pred_bbox, pred_obj, pred_cls, gt_boxes, gt_labels):
    labels = np.asarray(gt_labels).astype(np.int64)
    cls_ar = np.arange(C)
    in_maps = []
    for core in range(NCORES):
        bs = slice(core * BPC, (core + 1) * BPC)

        pairs = np.empty((P_PAIRS, W_PAIRS), np.float32)
        # boxes: [p, s, j=(b,k), c]
        pb = np.asarray(pred_bbox[bs, :M], np.float32).reshape(BPC, P_PAIRS, KP, 4)
        gb = np.asarray(gt_boxes[bs], np.float32).reshape(BPC, P_PAIRS, KP, 4)
        pairs[:, COL_PB:COL_PB + 32] = pb.transpose(1, 0, 2, 3).reshape(P_PAIRS, 32)
        pairs[:, COL_PB + 32:COL_PB + 64] = gb.transpose(1, 0, 2, 3).reshape(P_PAIRS, 32)
        cl = np.asarray(pred_cls[bs, :M], np.float32).reshape(BPC, P_PAIRS, KP, C)
        pairs[:, COL_CL:COL_CL + NPAIR * C] = cl.transpose(1, 0, 2, 3).reshape(
            P_PAIRS, NPAIR * C
        )
        lab = labels[bs].reshape(BPC, P_PAIRS, KP)
        onehot = (lab[..., None] == cls_ar).astype(np.float32)
        pairs[:, COL_MK:COL_MK + NPAIR * C] = onehot.transpose(1, 0, 2, 3).reshape(
            P_PAIRS, NPAIR * C
        )

        po = np.asarray(pred_obj[bs], np.float32)
        obj = np.full((120, F_OBJ), PAD, np.float32)
        obj[0:P_OBJ] = po.reshape(P_OBJ, F_OBJ)
        obj[P_OBJ:P_OBJ + BPC, 0:M] = -po[:, :M]
        obj[P_OBJ + BPC:P_OBJ + 2 * BPC, 0:M] = po[:, :M]

        in_maps.append({"pairs": pairs, "obj": obj})
    return in_maps


def finalize(per_core_partials):
    s_iou = s_ratio = s_all = s_pos = s_posplus = s_lse = s_picked = 0.0
    for p in per_core_partials:
        p = p.astype(np.float64)
        s_iou += p[:, 0].sum()
        s_ratio += p[:, 1].sum()
        s_all += p[0:P_OBJ, 2].sum()
        s_pos += p[P_OBJ:P_OBJ + BPC, 2].sum()
        s_posplus += p[P_OBJ + BPC:P_OBJ + 2 * BPC, 2].sum()
        s_lse += p[:, 3].sum()
        s_picked += p[:, 4].sum()
    n_pos = B * M
    n_neg = B * (N - M)
    loss_bbox = 5.0 * (n_pos - s_iou + s_ratio) / n_pos
    loss_obj = s_pos / n_pos + 0.5 * (s_all - s_posplus) / n_neg
    loss_cls = (s_lse - s_picked) / n_pos
    total = loss_bbox + loss_obj + loss_cls
    return np.array([total, loss_bbox, loss_obj, loss_cls], dtype=np.float32)


def kernel(pred_bbox, pred_obj, pred_cls, gt_boxes, gt_labels):
    from concourse.bass_utils import run_bass_kernel_spmd

    nc = build_bass()
    in_maps = make_in_maps(pred_bbox, pred_obj, pred_cls, gt_boxes, gt_labels)
    res = run_bass_kernel_spmd(nc, in_maps, core_ids=list(range(NCORES)))
    return finalize([r["partials"] for r in res.results])


# revision 8
# speedup vs baseline: 1.1454x; 1.0560x over previous
# Trainium2 Bass kernel for nn_DetectionLoss (B=32, N=25200, M=200, C=80).
#
# Strategy: pure data-parallel over batch (4 batches per core, 8 cores).
# The reference only reads pred_bbox[:, :M] and pred_cls[:, :M], so only
# those slices are shipped to the device. Each core computes per-partition
# partial sums of the four loss terms; the host does the final (tiny)
# cross-core reduction and mean/lambda arithmetic in float64.
#
# Device inputs per core (host-packed into device layout):
#   boxes [100, 64] f32:    pred|gt boxes, [p, s, j=(b,k), c] packed
#   clsmask [100, 1280] bf16: cols 0:640 cls logits, 640:1280 one-hot mask
#   obj   [120, 900] bf16:  rows 0:112 all 4*25200 obj logits (flat reshape),
#                           rows 112:116 -x of positives (softplus(-x) term),
#                           rows 116:120 +x of positives (correction term),
#                           positives rows padded with -30 (softplus == 0)
# DMAs are chunked across the three DMA queues so ACT compute overlaps the
# transfers. Output per core: partials [128, 8] f32:
#   col 0 sum(iou), col 1 sum((enclose-union)/(enclose+eps)),
#   col 2 softplus sums (split by partition range as above),
#   col 3 sum(logsumexp), col 4 sum(picked logit)

import numpy as np

B, N, M, C = 32, 25200, 200, 80
NCORES = 8
BPC = B // NCORES          # 4 batches per core
KP = 2                     # anchors per (partition, batch)
P_PAIRS = M // KP          # 100 partitions for pair-space tiles
NPAIR = BPC * KP           # 8 pairs per partition
P_OBJ, F_OBJ = 112, 900    # 4*25200 = 112*900
EPS = 1e-7
PAD = -30.0                # softplus(PAD) == 0 exactly in f32
W_CM = 2 * NPAIR * C       # 1280

_CACHED_NC = None


def _emit(nc, tc, mybir, boxes, clsmask, obj, out):
    f32 = mybir.dt.float32
    bf16 = mybir.dt.bfloat16
    Alu = mybir.AluOpType
    Act = mybir.ActivationFunctionType

    with tc.tile_pool(name="main", bufs=1) as pool:
        ACC = pool.tile([128, 8], f32, name="ACC")
        nc.vector.memset(ACC[:], 0.0)

        BX = pool.tile([P_PAIRS, 64], f32, name="BX")
        CM = pool.tile([P_PAIRS, W_CM], bf16, name="CM")
        OBJ = pool.tile([120, F_OBJ], bf16, name="OBJ")
        # Chunked DMAs across the three DMA-capable queues; small boxes first
        # so the DVE chain starts early, obj/cls chunks pipeline with ACT.
        nc.sync.dma_start(out=BX[:], in_=boxes.ap())
        nc.scalar.dma_start(out=OBJ[0:64], in_=obj.ap()[0:64])
        nc.sync.dma_start(out=OBJ[64:120], in_=obj.ap()[64:120])
        nc.gpsimd.dma_start(out=CM[0:64], in_=clsmask.ap()[0:64])
        nc.gpsimd.dma_start(out=CM[64:100], in_=clsmask.ap()[64:100])

        # ---------------- objectness softplus (two row-chunks) ----------------
        Eo = pool.tile([120, F_OBJ], f32, name="Eo")
        So = pool.tile([120, F_OBJ], f32, name="So")
        for r0, r1 in ((0, 64), (64, 120)):
            nc.scalar.activation(Eo[r0:r1], OBJ[r0:r1], Act.Exp)
        # ---------------- classification: exp of logits (two row-chunks) ------
        Ec = pool.tile([P_PAIRS, NPAIR, C], f32, name="Ec")
        sums = pool.tile([P_PAIRS, NPAIR], f32, name="sums")
        lse = pool.tile([P_PAIRS, NPAIR], f32, name="lse")
        prod = pool.tile([P_PAIRS, NPAIR * C], f32, name="prod")
        CLf = CM[:, 0:NPAIR * C]
        MKf = CM[:, NPAIR * C:W_CM]
        for r0, r1 in ((0, 64), (64, 100)):
            nc.scalar.activation(
                Ec[r0:r1].rearrange("p a c -> p (a c)"), CLf[r0:r1], Act.Exp
            )
        # Ln passes (same activation table as Exp after the table merge below)
        for r0, r1 in ((0, 64), (64, 120)):
            nc.scalar.activation(So[r0:r1], Eo[r0:r1], Act.Ln, bias=1.0,
                                 accum_out=ACC[r0:r1, 2:3])
        for r0, r1 in ((0, 64), (64, 100)):
            nc.vector.reduce_sum(out=sums[r0:r1], in_=Ec[r0:r1],
                                 axis=mybir.AxisListType.X)
            nc.scalar.activation(lse[r0:r1], sums[r0:r1], Act.Ln,
                                 accum_out=ACC[r0:r1, 3:4])
            nc.vector.scalar_tensor_tensor(
                prod[r0:r1], CLf[r0:r1], 1.0, MKf[r0:r1], Alu.mult, Alu.mult,
                accum_out=ACC[r0:r1, 4:5],
            )

        # ---------------- bbox GIoU term ----------------
        PB = BX[:].rearrange("p (s j c) -> p s j c", s=2, c=4)
        cxcy = PB[:, :, :, 0:2]
        wh = PB[:, :, :, 2:4]
        C1 = pool.tile([P_PAIRS, 2, NPAIR, 2], f32, name="C1")
        C2 = pool.tile([P_PAIRS, 2, NPAIR, 2], f32, name="C2")
        nc.vector.scalar_tensor_tensor(C1[:], wh, -0.5, cxcy, Alu.mult, Alu.add)
        nc.vector.scalar_tensor_tensor(C2[:], wh, 0.5, cxcy, Alu.mult, Alu.add)
        I1 = pool.tile([P_PAIRS, NPAIR, 2], f32, name="I1")
        I2 = pool.tile([P_PAIRS, NPAIR, 2], f32, name="I2")
        E1 = pool.tile([P_PAIRS, NPAIR, 2], f32, name="E1")
        E2 = pool.tile([P_PAIRS, NPAIR, 2], f32, name="E2")
        nc.vector.tensor_tensor(I1[:], C1[:, 0], C1[:, 1], Alu.max)
        nc.vector.tensor_tensor(I2[:], C2[:, 0], C2[:, 1], Alu.min)
        nc.vector.tensor_tensor(E1[:], C1[:, 0], C1[:, 1], Alu.min)
        nc.vector.tensor_tensor(E2[:], C2[:, 0], C2[:, 1], Alu.max)
        ID = pool.tile([P_PAIRS, NPAIR, 2], f32, name="ID")
        IDr = pool.tile([P_PAIRS, NPAIR, 2], f32, name="IDr")
        ED = pool.tile([P_PAIRS, NPAIR, 2], f32, name="ED")
        nc.vector.tensor_sub(ID[:], I2[:], I1[:])
        nc.vector.tensor_relu(IDr[:], ID[:])
        nc.vector.tensor_sub(ED[:], E2[:], E1[:])
        inter = pool.tile([P_PAIRS, NPAIR], f32, name="inter")
        encl = pool.tile([P_PAIRS, NPAIR], f32, name="encl")
        nc.vector.tensor_mul(inter[:], IDr[:, :, 0], IDr[:, :, 1])
        nc.vector.tensor_mul(encl[:], ED[:, :, 0], ED[:, :, 1])
        A = pool.tile([P_PAIRS, 2, NPAIR], f32, name="A")
        nc.vector.tensor_mul(A[:], PB[:, :, :, 2], PB[:, :, :, 3])
        asum = pool.tile([P_PAIRS, NPAIR], f32, name="asum")
        nc.vector.tensor_add(asum[:], A[:, 0], A[:, 1])
        U = pool.tile([P_PAIRS, NPAIR], f32, name="U")
        nc.vector.scalar_tensor_tensor(U[:], inter[:], -1.0, asum[:],
                                       Alu.mult, Alu.add)
        Ue = pool.tile([P_PAIRS, NPAIR], f32, name="Ue")
        Ur = pool.tile([P_PAIRS, NPAIR], f32, name="Ur")
        nc.vector.tensor_scalar_add(Ue[:], U[:], EPS)
        nc.vector.reciprocal(Ur[:], Ue[:])
        # NOTE: tensor_tensor_reduce wedges the device (NRT_EXEC_UNIT_UNRECOVERABLE)
        # on this runtime; scalar_tensor_tensor's accum_out path works.
        t8a = pool.tile([P_PAIRS, NPAIR], f32, name="t8a")
        nc.vector.scalar_tensor_tensor(
            t8a[:], inter[:], 1.0, Ur[:], Alu.mult, Alu.mult,
            accum_out=ACC[0:P_PAIRS, 0:1],
        )
        EmU = pool.tile([P_PAIRS, NPAIR], f32, name="EmU")
        Ee = pool.tile([P_PAIRS, NPAIR], f32, name="Ee")
        Er = pool.tile([P_PAIRS, NPAIR], f32, name="Er")
        nc.vector.tensor_sub(EmU[:], encl[:], U[:])
        nc.vector.tensor_scalar_add(Ee[:], encl[:], EPS)
        nc.vector.reciprocal(Er[:], Ee[:])
        t8b = pool.tile([P_PAIRS, NPAIR], f32, name="t8b")
        nc.vector.scalar_tensor_tensor(
            t8b[:], EmU[:], 1.0, Er[:], Alu.mult, Alu.mult,
            accum_out=ACC[0:P_PAIRS, 1:2],
        )

        nc.sync.dma_start(out=out.ap(), in_=ACC[:])


def build_bass():
    global _CACHED_NC
    if _CACHED_NC is not None:
        return _CACHED_NC
    import concourse.bacc as bacc
    import concourse.tile as tile
    import concourse.mybir as mybir

    f32 = mybir.dt.float32
    bf16 = mybir.dt.bfloat16
    Act = mybir.ActivationFunctionType

    class FastTileContext(tile.TileContext):
        # Same as TileContext._drain_and_barrier but: sem-only barrier and no
        # trailing second barrier — trims the kernel-tail cost.
        def _drain_and_barrier(self, tick_clock, wait_clock):
            drain_inst = self.nc.sync.drain()
            wait_clock.add_sem_waits(
                drain_inst.ins, tile.ScopedClock({None: tick_clock.global_clock})
            )
            self.nc.all_engine_barrier(sem_only=True)
            popped = self.nc._tile_sem_poison_stack.pop()
            assert popped is self._sem_poison
            self.nc.clear_and_free_semaphores(list(self.sems.allocated().values()))

    nc = bacc.Bacc("TRN2", target_bir_lowering=False, debug=False,
                   num_devices=NCORES)
    boxes = nc.dram_tensor("boxes", [P_PAIRS, 64], f32, kind="ExternalInput")
    clsmask = nc.dram_tensor("clsmask", [P_PAIRS, W_CM], bf16,
                             kind="ExternalInput")
    obj = nc.dram_tensor("obj", [120, F_OBJ], bf16, kind="ExternalInput")
    out = nc.dram_tensor("partials", [128, 8], f32, kind="ExternalOutput")
    with FastTileContext(nc) as tc:
        _emit(nc, tc, mybir, boxes, clsmask, obj, out)

    # Route every Exp/Ln to the one table that holds both, so the kernel pays
    # a single ACT_TABLE_LOAD instead of ping-ponging between per-func tables.
    # Patch is scoped to this compile; table ids are positional so only the
    # membership sets are altered (ids stay valid).
    orig_tables = bacc.get_activation_tables

    def _merged_tables(arch):
        out_d = {}
        for name, s in orig_tables(arch).items():
            s2 = set(s)
            if name != "natural_log_exp_and_others":
                s2.discard(Act.Exp)
                s2.discard(Act.Ln)
            out_d[name] = s2
        return out_d

    bacc.get_activation_tables = _merged_tables
    try:
        nc.compile()
    finally:
        bacc.get_activation_tables = orig_tables
    _CACHED_NC = nc
    return nc


def make_in_maps(pred_bbox, pred_obj, pred_cls, gt_boxes, gt_labels):
    import ml_dtypes

    bf16 = ml_dtypes.bfloat16
    labels = np.asarray(gt_labels).astype(np.int64)
    cls_ar = np.arange(C)
    in_maps = []
    for core in range(NCORES):
        bs = slice(core * BPC, (core + 1) * BPC)

        boxes = np.empty((P_PAIRS, 64), np.float32)
        pb = np.asarray(pred_bbox[bs, :M], np.float32).reshape(BPC, P_PAIRS, KP, 4)
        gb = np.asarray(gt_boxes[bs], np.float32).reshape(BPC, P_PAIRS, KP, 4)
        boxes[:, 0:32] = pb.transpose(1, 0, 2, 3).reshape(P_PAIRS, 32)
        boxes[:, 32:64] = gb.transpose(1, 0, 2, 3).reshape(P_PAIRS, 32)

        clsmask = np.empty((P_PAIRS, W_CM), bf16)
        cl = np.asarray(pred_cls[bs, :M], np.float32).reshape(BPC, P_PAIRS, KP, C)
        clsmask[:, 0:NPAIR * C] = cl.transpose(1, 0, 2, 3).reshape(
            P_PAIRS, NPAIR * C
        ).astype(bf16)
        lab = labels[bs].reshape(BPC, P_PAIRS, KP)
        onehot = (lab[..., None] == cls_ar).astype(np.float32)
        clsmask[:, NPAIR * C:W_CM] = onehot.transpose(1, 0, 2, 3).reshape(
            P_PAIRS, NPAIR * C
        ).astype(bf16)

        po = np.asarray(pred_obj[bs], np.float32)
        obj = np.full((120, F_OBJ), PAD, np.float32)
        obj[0:P_OBJ] = po.reshape(P_OBJ, F_OBJ)
        obj[P_OBJ:P_OBJ + BPC, 0:M] = -po[:, :M]
        obj[P_OBJ + BPC:P_OBJ + 2 * BPC, 0:M] = po[:, :M]

        in_maps.append({"boxes": boxes, "clsmask": clsmask,
                        "obj": obj.astype(bf16)})
    return in_maps


def finalize(per_core_partials):
    s_iou = s_ratio = s_all = s_pos = s_posplus = s_lse = s_picked = 0.0
    for p in per_core_partials:
        p = p.astype(np.float64)
        s_iou += p[:, 0].sum()
        s_ratio += p[:, 1].sum()
        s_all += p[0:P_OBJ, 2].sum()
        s_pos += p[P_OBJ:P_OBJ + BPC, 2].sum()
        s_posplus += p[P_OBJ + BPC:P_OBJ + 2 * BPC, 2].sum()
        s_lse += p[:, 3].sum()
        s_picked += p[:, 4].sum()
    n_pos = B * M
    n_neg = B * (N - M)
    loss_bbox = 5.0 * (n_pos - s_iou + s_ratio) / n_pos
    loss_obj = s_pos / n_pos + 0.5 * (s_all - s_posplus) / n_neg
    loss_cls = (s_lse - s_picked) / n_pos
    total = loss_bbox + loss_obj + loss_cls
    return np.array([total, loss_bbox, loss_obj, loss_cls], dtype=np.float32)


def kernel(pred_bbox, pred_obj, pred_cls, gt_boxes, gt_labels):
    from concourse.bass_utils import run_bass_kernel_spmd

    nc = build_bass()
    in_maps = make_in_maps(pred_bbox, pred_obj, pred_cls, gt_boxes, gt_labels)
    res = run_bass_kernel_spmd(nc, in_maps, core_ids=list(range(NCORES)))
    return finalize([r["partials"] for r in res.results])
